# revision 6
# baseline (speedup 1.0000x reference)
"""Trainium2 Bass kernel v2 for GQA attention (B=2, L=2048, D=3072, H=24,
KV=8, HD=128, causal, half-split RoPE).

Sharding: TP=4 over heads x DP=2 over batch on 8 NeuronCores.
Core c = 4*b + s handles batch b with q-heads 6s..6s+5 and kv-heads 2s,2s+1.
Each core computes a partial o_proj output [L, D]; the host sums the 4 TP
partials per batch.

v2 structure (vs v1): single fused per-quarter pipeline.
  quarter qt: DMA xt half-tiles -> Q/K proj (rope fused) -> V proj ->
  attention block b=qt (6 heads) with o_proj(qt-1) matmuls interleaved
  into the chunk loop (fills PE while ACT runs exp), o_proj psum groups
  drain to HBM as they complete. Quarter 0 projects d-major across 8
  PSUM accumulators so PE starts as soon as the first weight/x chunks
  arrive. o_proj(3) drains serially at the end.
"""

import numpy as np
import ml_dtypes

import concourse.mybir as mybir
import concourse.tile as tile
from concourse import bacc
from concourse.bass_utils import run_bass_kernel_spmd

BF16NP = ml_dtypes.bfloat16

B, L, D = 2, 2048, 3072
H, KV, HD = 24, 8, 128
GROUP = H // KV          # 3
THETA = 500000.0
SCALE = HD ** -0.5
N_CORES = 8
TP = 4                   # tensor-parallel over heads
NQH = H // TP            # 6 q heads per core
NKH = KV // TP           # 2 kv heads per core
QCOLS = NQH * HD         # 768
KCOLS = NKH * HD         # 256
ND = D // 128            # 24 contraction chunks
NLT = L // 128           # 16 l-tiles
NB = L // 512            # 4 q-blocks
NE = D // 512            # 6 output col blocks
BF = mybir.dt.bfloat16
F32 = mybir.dt.float32


def _ls(i, w=512):
    return slice(i * w, (i + 1) * w)


def _rope_tables():
    half = HD // 2
    inv_freq = 1.0 / (THETA ** (np.arange(half, dtype=np.float64) / half))
    ang = np.arange(L, dtype=np.float64)[:, None] * inv_freq[None, :]  # [L, 64]
    cosT = np.cos(ang).T.astype(np.float32)   # [64, L]
    sinT = np.sin(ang).T.astype(np.float32)
    cosF = np.concatenate([cosT, cosT], 0)    # [128, L]
    sinF = np.concatenate([-sinT, sinT], 0)   # rows 0:64 get -sin
    return cosF.astype(BF16NP), sinF.astype(BF16NP)


def _mask_tiles():
    # one upper-triangle [128,128] band: within diagonal chunk m the only
    # partially-masked q-band is [128m, 128m+128) where allowed iff c' >= r
    r = np.arange(128)[:, None]
    c = np.arange(128)[None, :]
    return (c >= r).astype(BF16NP)  # [128, 128]


def _emit(nc, phases=(1, 2, 3)):
    xT = nc.dram_tensor("xT", [D, L], BF, kind="ExternalInput")
    wqk = nc.dram_tensor("wqk", [D, QCOLS + KCOLS], BF, kind="ExternalInput")
    wv = nc.dram_tensor("wv", [D, KCOLS], BF, kind="ExternalInput")
    wo = nc.dram_tensor("wo", [QCOLS, D], BF, kind="ExternalInput")
    out = nc.dram_tensor("out", [L, D], BF, kind="ExternalOutput")

    cosF, sinF = _rope_tables()
    cosc = nc.inline_tensor(np.ascontiguousarray(cosF), name="cosc")
    sinc = nc.inline_tensor(np.ascontiguousarray(sinF), name="sinc")
    maskc = nc.inline_tensor(np.ascontiguousarray(_mask_tiles()), name="maskc")

    Exp = mybir.ActivationFunctionType.Exp

    with tile.TileContext(nc) as tc:
        with (
            tc.tile_pool(name="persist", bufs=1) as P,
            tc.tile_pool(name="wres", bufs=1) as WR,
            tc.tile_pool(name="xt", bufs=1) as XT,
            tc.tile_pool(name="qtq", bufs=1) as QTQ,
            tc.tile_pool(name="otq", bufs=2) as OTQ,
            tc.tile_pool(name="ropet", bufs=1) as RT,
            tc.tile_pool(name="p2", bufs=2) as P2,
            tc.tile_pool(name="stage", bufs=6) as SG,
            tc.tile_pool(name="ps_qk", bufs=2, space="PSUM") as PQ,
            tc.tile_pool(name="ps_sc", bufs=2, space="PSUM") as PS,
            tc.tile_pool(name="ps_o", bufs=2, space="PSUM") as PO,
            tc.tile_pool(name="ps_op", bufs=2, space="PSUM") as POP,
        ):
            cos_sb = P.tile([128, L], BF, tag="cos")
            nc.sync.dma_start(out=cos_sb, in_=cosc.ap())
            sin_sb = P.tile([128, L], BF, tag="sin")
            nc.gpsimd.dma_start(out=sin_sb, in_=sinc.ap())
            ones_sb = P.tile([128, 128], F32, tag="ones")
            nc.vector.memset(ones_sb, 1.0)

            kT_sb = [
                P.tile([128, L], BF, tag=f"kT{i}", name=f"kT{i}")
                for i in range(NKH)
            ]
            v_sb = P.tile([128, NLT, KCOLS], BF, tag="vsb")
            wo_sb = P.tile([128, NQH, D], BF, tag="wosb")

            wqk_sb = WR.tile([128, ND, QCOLS + KCOLS], BF, tag="wqksb")
            wqk_r = wqk.ap().rearrange("(dc p) n -> p dc n", p=128)
            wv_sb = WR.tile([128, ND, KCOLS], BF, tag="wvsb")
            wv_r = wv.ap().rearrange("(dc p) n -> p dc n", p=128)
            mask_sb = WR.tile([128, 128], BF, tag="mask")
            xT_r = xT.ap().rearrange("(dc p) l -> p dc l", p=128)
            wo_r = wo.ap().rearrange("(c p) n -> p c n", p=128)
            out_r = out.ap().rearrange(
                "(lt p) (et n) -> p lt et n", p=128, n=512
            )

            # --- o_proj machinery: emitted lazily as "thunks" so the PE
            # work interleaves into the following quarter's attention.
            def oproj_thunks(oT_q, qt):
                """Yield one closure per PE matmul for o_proj of quarter qt;
                the group-final closure also emits the copy + store."""
                for g, (lt_l, e) in enumerate(
                    (i, j) for i in range(4) for j in range(NE)
                ):
                    lt = 4 * qt + lt_l
                    pp = [None]

                    def mk(c, pp=pp, lt_l=lt_l, lt=lt, e=e, g=g):
                        def thunk():
                            if c == 0:
                                pool, tg = ((POP, "pp"), (PQ, "psqk"))[g % 2]
                                pp[0] = pool.tile(
                                    [128, 512], F32, tag=tg,
                                    name=f"pp_{qt}_{g}",
                                )
                            nc.tensor.matmul(
                                pp[0],
                                lhsT=oT_q[:, c, lt_l * 128:(lt_l + 1) * 128],
                                rhs=wo_sb[:, c, _ls(e)],
                                start=(c == 0),
                                stop=(c == NQH - 1),
                            )
                            if c == NQH - 1:
                                st = SG.tile([128, 512], BF, tag="st",
                                             name=f"st_{qt}_{g}")
                                # Pool cannot read PSUM; split the PSUM
                                # drain copies between DVE and ACT
                                if g % 2:
                                    nc.scalar.copy(st, pp[0])
                                else:
                                    nc.vector.tensor_copy(st, pp[0])
                                if drain_mode[0]:
                                    oeng = (nc.sync, nc.scalar, nc.gpsimd)[g % 3]
                                else:
                                    oeng = (nc.sync, nc.scalar)[g % 2]
                                oeng.dma_start(out=out_r[:, lt, e, :], in_=st)
                        return thunk

                    for c in range(NQH):
                        yield mk(c)

            pending = iter(())  # o_proj thunks of the previous quarter
            drain_mode = [False]

            def pull(n=1):
                for _ in range(n):
                    th = next(pending, None)
                    if th is None:
                        return
                    th()

            LQ = 512
            for qt in range(NB):
                hs = qt * LQ
                # --- xt DMA (single full-width tile per quarter;
                # quarters 1-3 prefetch during the previous attention) ---
                xt_sb = XT.tile([128, ND, LQ], BF, tag="xt",
                                name=f"xt_{qt}")
                if qt == 0:
                    # startup: interleave weight + x chunk loads so the
                    # d-major boot matmuls start on first-chunk arrival
                    for d in range(ND):
                        eng = (nc.scalar, nc.sync, nc.gpsimd)[d % 3]
                        eng.dma_start(out=wqk_sb[:, d, :], in_=wqk_r[:, d, :])
                        xeng = (nc.sync, nc.gpsimd)[d % 2]
                        xeng.dma_start(
                            out=xt_sb[:, d, :], in_=xT_r[:, d, hs:hs + LQ]
                        )
                    for d in range(0, ND, 3):
                        nc.scalar.dma_start(
                            out=wv_sb[:, d:d + 3, :], in_=wv_r[:, d:d + 3, :]
                        )
                    nc.sync.dma_start(out=mask_sb, in_=maskc.ap())
                else:
                    for g in range(ND // 3):
                        eng = (nc.sync, nc.scalar)[g % 2]
                        eng.dma_start(
                            out=xt_sb[:, 3 * g:3 * g + 3, :],
                            in_=xT_r[:, 3 * g:3 * g + 3, hs:hs + LQ],
                        )
                    if qt == 1 and 3 in phases:
                        # after xt(1) so the scalar ring stays clear for it;
                        # needed from attention(1) onwards
                        for c in range(NQH):
                            nc.scalar.dma_start(
                                out=wo_sb[:, c, :], in_=wo_r[:, c, :]
                            )

                # --- Q/K projection (rope fused) ---
                qTq = QTQ.tile([128, NQH, LQ], BF, tag="qTq",
                               name=f"qTq_{qt}")

                def rope(src_ap, mi):
                    qkb = RT.tile([128, 512], BF, tag="qkb",
                                  name=f"qkb_{qt}_{mi}")
                    nc.vector.tensor_copy(qkb, src_ap)
                    rot = RT.tile([128, 512], BF, tag="rot",
                                  name=f"rot_{qt}_{mi}")
                    nc.vector.tensor_copy(out=rot[0:64, :], in_=qkb[64:128, :])
                    nc.vector.tensor_copy(out=rot[64:128, :], in_=qkb[0:64, :])
                    t1 = RT.tile([128, 512], BF, tag="t1",
                                 name=f"t1_{qt}_{mi}")
                    nc.vector.tensor_mul(t1, qkb, cos_sb[:, hs:hs + LQ])
                    nc.vector.tensor_mul(rot, rot, sin_sb[:, hs:hs + LQ])
                    dst = (qTq[:, mi, :] if mi < NQH
                           else kT_sb[mi - NQH][:, hs:hs + LQ])
                    nc.vector.tensor_add(dst, t1, rot)

                if qt == 0:
                    # d-major boot: 8 full-width accumulators across the 4
                    # PSUM pools; matmuls chase DMA chunk arrivals.
                    pools = (PQ, PS, PO, POP)
                    accs = [
                        pools[mi % 4].tile([128, 512], F32,
                                           tag=("psqk", "sc", "po", "pp")[mi % 4],
                                           name=f"boot_{mi}")
                        for mi in range(NQH + NKH)
                    ]
                    for d in range(ND):
                        for mi in range(NQH + NKH):
                            nc.tensor.matmul(
                                accs[mi],
                                lhsT=wqk_sb[:, d, mi * 128:(mi + 1) * 128],
                                rhs=xt_sb[:, d, :],
                                start=(d == 0),
                                stop=(d == ND - 1),
                            )
                    for mi in range(NQH + NKH):
                        rope(accs[mi], mi)
                    accs = None
                else:
                    for mi in range(NQH + NKH):
                        ps = PQ.tile([128, 512], F32, tag="psqk",
                                     name=f"ps_{qt}_{mi}")
                        for d in range(ND):
                            nc.tensor.matmul(
                                ps,
                                lhsT=wqk_sb[:, d, mi * 128:(mi + 1) * 128],
                                rhs=xt_sb[:, d, :],
                                start=(d == 0),
                                stop=(d == ND - 1),
                            )
                        rope(ps, mi)

                # --- V projection (natural layout) ---
                for lt in range(4):
                    glt = qt * 4 + lt
                    pv = PS.tile([128, 512], F32, tag="sc",
                                 name=f"pv_{qt}_{lt}")
                    for d in range(ND):
                        nc.tensor.matmul(
                            pv[:, 0:KCOLS],
                            lhsT=xt_sb[:, d, lt * 128:(lt + 1) * 128],
                            rhs=wv_sb[:, d, :],
                            start=(d == 0),
                            stop=(d == ND - 1),
                        )
                    nc.vector.tensor_copy(v_sb[:, glt, :], pv[:, 0:KCOLS])

                if 2 not in phases:
                    continue
                # --- attention for q-block b=qt, all heads, with o_proj
                # thunks of quarter qt-1 interleaved one per chunk ---
                b = qt
                nch = 4 * (b + 1)
                oT_q = OTQ.tile([128, NQH, LQ], BF, tag="oTq",
                                name=f"oTq_{qt}")
                flat = [(h, j) for h in range(NQH) for j in range(nch)]
                sc_map = {}

                def emit_S(h, j):
                    kv = h // GROUP
                    m = j - 4 * b
                    col0 = 128 * m if m > 0 else 0
                    sc = PS.tile([128, 512], F32, tag="sc",
                                 name=f"sc_{b}_{h}_{j}")
                    nc.tensor.matmul(
                        sc[:, col0:512],
                        lhsT=kT_sb[kv][:, j * 128:(j + 1) * 128],
                        rhs=qTq[:, h, col0:512],
                        start=True,
                        stop=True,
                    )
                    sc_map[(h, j)] = sc

                po_h = {}
                acc_h = {}
                fin = []  # deferred head finalizers

                def finalize(h):
                    # partition-reduce the denominators, normalize; emitted
                    # one chunk into the NEXT head so the PE psm matmul
                    # never stalls on the DVE accumulator chain.
                    psm = PQ.tile([128, 512], F32, tag="psqk",
                                  name=f"psm_{qt}_{h}")
                    nc.tensor.matmul(
                        psm, lhsT=ones_sb, rhs=acc_h[h], start=True, stop=True
                    )
                    rc = P2.tile([128, 512], F32, tag="rc",
                                 name=f"rc_{qt}_{h}")
                    nc.vector.reciprocal(rc, psm)
                    nc.vector.tensor_mul(oT_q[:, h, :], po_h[h], rc)
                    del po_h[h], acc_h[h]

                emit_S(*flat[0])
                for idx, (h, j) in enumerate(flat):
                    kv = h // GROUP
                    if idx + 1 < len(flat):
                        emit_S(*flat[idx + 1])
                    if j == 0:
                        po_h[h] = PO.tile([128, 512], F32, tag="po",
                                          name=f"po_{qt}_{h}")
                        acc_h[h] = P2.tile([128, 512], F32, tag="acc",
                                           name=f"acc_{qt}_{h}")
                    m = j - 4 * b
                    col0 = 128 * m if m > 0 else 0
                    pt = P2.tile([128, 512], BF, tag="pt", bufs=4,
                                 name=f"pt_{b}_{h}_{j}")
                    if col0:
                        # dead q-cols of this diagonal chunk: zero them so
                        # the full-width AV matmul adds nothing there
                        nc.gpsimd.memset(pt[:, 0:col0], 0.0)
                    nc.scalar.activation(pt[:, col0:512],
                                         sc_map.pop((h, j))[:, col0:512],
                                         Exp, scale=SCALE)
                    if m >= 0:
                        # triangular band of the diagonal chunk
                        nc.vector.tensor_mul(pt[:, col0:col0 + 128],
                                             pt[:, col0:col0 + 128], mask_sb)
                    # denominator partials accumulate on DVE (fp32); the
                    # initial copy runs on Pool (SBUF->SBUF is legal there)
                    if j == 0:
                        nc.vector.tensor_copy(acc_h[h], pt)
                    else:
                        nc.vector.tensor_add(acc_h[h][:, col0:512],
                                             acc_h[h][:, col0:512],
                                             pt[:, col0:512])
                    if fin:
                        fin.pop()()
                    pull(2)  # o_proj matmuls of quarter qt-1
                    nc.tensor.matmul(
                        po_h[h],
                        lhsT=v_sb[:, j, kv * 128:(kv + 1) * 128],
                        rhs=pt,
                        start=(j == 0),
                        stop=(j == nch - 1),
                    )
                    if j == nch - 1:
                        fin.append(lambda h=h: finalize(h))
                while fin:
                    fin.pop()()

                if 3 in phases:
                    drain_mode[0] = True
                    pull(10 ** 6)  # drain any o_proj leftovers of qt-1
                    drain_mode[0] = False
                    pending = oproj_thunks(oT_q, qt)

            if 3 in phases:
                drain_mode[0] = True
                pull(10 ** 6)  # o_proj of the final quarter
    return nc


_NC_CACHE = {}


def build(phases=(1, 2, 3)):
    key = tuple(phases)
    if key not in _NC_CACHE:
        nc = bacc.Bacc(
            "TRN2", target_bir_lowering=False, debug=False, num_devices=N_CORES
        )
        _emit(nc, phases)
        nc.compile()
        _NC_CACHE[key] = nc
    return _NC_CACHE[key]


def prep_in_maps(x, Wq, Wk, Wv, Wo):
    """Shard + cast + layout the full inputs into 8 per-core input maps."""
    x = np.asarray(x)
    Wq, Wk, Wv, Wo = (np.asarray(a) for a in (Wq, Wk, Wv, Wo))
    in_maps = []
    wqk_s = [
        np.ascontiguousarray(np.hstack([
            Wq[:, s * QCOLS:(s + 1) * QCOLS],
            Wk[:, s * KCOLS:(s + 1) * KCOLS],
        ])).astype(BF16NP)
        for s in range(TP)
    ]
    wv_s = [np.ascontiguousarray(Wv[:, s * KCOLS:(s + 1) * KCOLS]).astype(BF16NP)
            for s in range(TP)]
    wo_s = [np.ascontiguousarray(Wo[s * QCOLS:(s + 1) * QCOLS, :]).astype(BF16NP)
            for s in range(TP)]
    xT_b = [np.ascontiguousarray(x[b].T).astype(BF16NP) for b in range(B)]
    for core in range(N_CORES):
        b, s = divmod(core, TP)
        in_maps.append({
            "xT": xT_b[b],
            "wqk": wqk_s[s],
            "wv": wv_s[s],
            "wo": wo_s[s],
        })
    return in_maps


def kernel(x, Wq, Wk, Wv, Wo):
    nc = build()
    in_maps = prep_in_maps(x, Wq, Wk, Wv, Wo)
    res = run_bass_kernel_spmd(nc, in_maps, list(range(N_CORES)))
    out = np.zeros((B, L, D), np.float32)
    for core in range(N_CORES):
        b, _s = divmod(core, TP)
        out[b] += res.results[core]["out"].astype(np.float32)
    return out
